# revision 1
# baseline (speedup 1.0000x reference)
"""Causal self-attention (B=2, S=2048, D=1024, H=16, Dh=64) on 8 NeuronCores.

Sharding: core c -> batch b = c//4, head-group g = c%4 (heads 4g..4g+3).
Each core computes QKV projection for its 4 heads, causal attention
(scores kept transposed: [k, q] layout so no on-chip transposes are
needed), and a partial output projection over its local head dims.
Host sums the 4 partials per batch and adds b_out.

All matmuls run as float32r (full-rate PE, fp32 storage).
"""

import numpy as np
from contextlib import ExitStack

B = 2
S = 2048
D = 1024
NH = 16
DH = 64
N_CORES = 8
HPC = 4            # heads per core
EL = HPC * DH      # 256 local head dims per core
KD = D // 128      # 8 contraction chunks for projections
KT = S // 128      # 16 key tiles

_NC = None
_last_in_maps = None


def _build_program():
    import concourse.mybir as mybir
    import concourse.tile as tile
    from concourse import bacc

    F32 = mybir.dt.float32
    F32R = mybir.dt.float32r
    Exp = mybir.ActivationFunctionType.Exp

    def r32(ap):
        return ap.bitcast(F32R)

    nc = bacc.Bacc("TRN2", target_bir_lowering=False, debug=False,
                   num_devices=N_CORES)

    xt_d = nc.dram_tensor("xt", [D, S], F32R, kind="ExternalInput")
    wqk_d = nc.dram_tensor("wqk", [D, 2 * EL], F32R, kind="ExternalInput")
    wv_d = nc.dram_tensor("wv", [D, HPC * 65], F32R, kind="ExternalInput")
    bqk_d = nc.dram_tensor("bqk", [128, 4], F32, kind="ExternalInput")
    ones_d = nc.dram_tensor("ones", [1, 512], F32R, kind="ExternalInput")
    bv_d = nc.dram_tensor("bv", [1, HPC * 65], F32R, kind="ExternalInput")
    wo_d = nc.dram_tensor("wo", [EL, D], F32R, kind="ExternalInput")
    out_d = nc.dram_tensor("out", [S, D], F32, kind="ExternalOutput")

    with nc.allow_low_precision(reason="fp32r matmul input tensors"), \
         tile.TileContext(nc) as tc, ExitStack() as ctx:
        const = ctx.enter_context(tc.tile_pool(name="const", bufs=1))
        work = ctx.enter_context(tc.tile_pool(name="work", bufs=1))

        wo_s = [const.tile([128, D], F32R, tag=f"wo{i}", name=f"wo{i}") for i in range(2)]
        for i in range(2):
            nc.sync.dma_start(out=wo_s[i], in_=wo_d[128 * i:128 * (i + 1), :])
        bqk_s = const.tile([128, 4], F32, tag="bqk", name="bqk")
        nc.sync.dma_start(out=bqk_s, in_=bqk_d[:, :])
        bv_s = const.tile([1, HPC * 65], F32R, tag="bv", name="bv")
        nc.sync.dma_start(out=bv_s, in_=bv_d[:, :])
        ones_s = const.tile([1, 512], F32R, tag="ones", name="ones")
        nc.sync.dma_start(out=ones_s, in_=ones_d[:, :])
        # ones living on partition 64, for the base-64 denominator matmul
        ones64_s = const.tile([65, 64], F32R, tag="ones64", name="ones64")
        nc.sync.dma_start(out=ones64_s[64:65, :], in_=ones_d[0:1, 0:64])

        # qkT: e-tiles 0,1 = Q (heads 0..3), 2,3 = K
        qk_s = [work.tile([128, S], F32R, tag=f"qk{e}", name=f"qk{e}") for e in range(4)]
        # V augmented: per key-tile [128, 4*65]; col 64 of each head = 1.0
        vaug_s = [work.tile([128, HPC * 65], F32R, tag=f"va{t}", name=f"va{t}") for t in range(KT)]
        # normalized attn output, transposed: [d_local, s]
        attnT_s = [work.tile([128, S], F32R, tag=f"at{d}", name=f"at{d}") for d in range(2)]

        # ---------------- Phase A: projections ----------------
        with ExitStack() as actx:
            pin = actx.enter_context(tc.tile_pool(name="pin", bufs=1))
            psA = actx.enter_context(tc.tile_pool(name="psA", bufs=2, space="PSUM"))
            psV = actx.enter_context(tc.tile_pool(name="psV", bufs=2, space="PSUM"))

            xt_s = [pin.tile([128, S], F32R, tag=f"xt{k}", name=f"xt{k}") for k in range(KD)]
            wqk_s = [pin.tile([128, 2 * EL], F32R, tag=f"wqk{k}", name=f"wqk{k}") for k in range(KD)]
            wv_s = [pin.tile([128, HPC * 65], F32R, tag=f"wv{k}", name=f"wv{k}") for k in range(KD)]
            for k in range(KD):
                nc.sync.dma_start(out=wqk_s[k], in_=wqk_d[128 * k:128 * (k + 1), :])
                nc.sync.dma_start(out=wv_s[k], in_=wv_d[128 * k:128 * (k + 1), :])
                nc.sync.dma_start(out=xt_s[k], in_=xt_d[128 * k:128 * (k + 1), :])

            # qkT[e, s] = sum_d wqk[d, e] * xt[d, s]   (+ bias per-partition)
            for e in range(4):
                for sc in range(4):
                    ps = psA.tile([128, 512], F32, tag="psA", name="psA")
                    for k in range(KD):
                        nc.tensor.matmul(
                            ps,
                            lhsT=r32(wqk_s[k][:, 128 * e:128 * (e + 1)]),
                            rhs=r32(xt_s[k][:, 512 * sc:512 * (sc + 1)]),
                            start=(k == 0), stop=(k == KD - 1))
                    nc.vector.tensor_scalar_add(
                        out=qk_s[e][:, 512 * sc:512 * (sc + 1)], in0=ps,
                        scalar1=bqk_s[:, e:e + 1])

            # Vaug[s, hc] = sum_d xt[d, s] * wv[d, hc]  (+ bias via ones row;
            # wv col 65h+64 is zero with bias 1.0 -> the softmax-denominator ones)
            for st in range(KT):
                ps = psV.tile([128, HPC * 65], F32, tag="psV", name="psV")
                for k in range(KD):
                    nc.tensor.matmul(
                        ps,
                        lhsT=r32(xt_s[k][:, 128 * st:128 * (st + 1)]),
                        rhs=r32(wv_s[k]),
                        start=(k == 0), stop=False)
                nc.tensor.matmul(ps, lhsT=r32(ones_s[0:1, 0:128]), rhs=r32(bv_s),
                                 start=False, stop=True)
                nc.vector.tensor_copy(vaug_s[st], ps)

        # ---------------- Phase B: attention ----------------
        # Head pairs: K=64 score matmuls strictly alternate row groups
        # (bases 0/64) so they run concurrently and keep the PE clock warm.
        with ExitStack() as bctx:
            psS = bctx.enter_context(tc.tile_pool(name="psS", bufs=1, space="PSUM"))
            psOT = bctx.enter_context(tc.tile_pool(name="psOT", bufs=1, space="PSUM"))
            pp = bctx.enter_context(tc.tile_pool(name="pp", bufs=3))
            rp = bctx.enter_context(tc.tile_pool(name="rp", bufs=2))
            tn = bctx.enter_context(tc.tile_pool(name="tn", bufs=2))

            for hp in range(2):
                qt = qk_s[hp]
                kt_ = qk_s[2 + hp]
                for qh in range(2):
                    ot = {}
                    for h2 in range(2):
                        for jq in range(2):
                            ot[(h2, jq)] = psOT.tile(
                                [65, 512], F32, tag=f"ot{h2}{jq}", name=f"ot{h2}{jq}")
                    for ki in range(8 * (qh + 1)):
                        s_ps = {}
                        for h2 in range(2):
                            s_ps[h2] = psS.tile([128, 1024], F32,
                                                tag=f"s{h2}", name=f"s{h2}")
                        for qq in range(2):
                            q0 = 1024 * qh + 512 * qq
                            if q0 + 512 <= 128 * ki:
                                continue
                            for h2 in range(2):
                                pb = 64 * h2
                                nc.tensor.matmul(
                                    s_ps[h2][:, 512 * qq:512 * (qq + 1)],
                                    lhsT=r32(kt_[pb:pb + 64, 128 * ki:128 * (ki + 1)]),
                                    rhs=r32(qt[pb:pb + 64, q0:q0 + 512]),
                                    start=True, stop=True)
                        for h2 in range(2):
                            alo = max(1024 * qh, 128 * ki)
                            p_t = pp.tile([128, 1024], F32R, tag="p", name="pt")
                            nc.scalar.activation(
                                out=p_t[:, alo - 1024 * qh:1024],
                                in_=s_ps[h2][:, alo - 1024 * qh:1024],
                                func=Exp, scale=0.125)
                            if 128 * ki >= 1024 * qh:
                                off = 128 * ki - 1024 * qh
                                nc.gpsimd.affine_select(
                                    out=p_t[:, off:off + 128],
                                    in_=p_t[:, off:off + 128],
                                    compare_op=mybir.AluOpType.is_ge, fill=0.0,
                                    base=0, pattern=[[1, 128]], channel_multiplier=-1)
                            h = 2 * hp + h2
                            for jq in range(2):
                                jq_g = 2 * qh + jq
                                if ki > 4 * jq_g + 3:
                                    continue
                                q0g = max(512 * jq_g, 128 * ki)
                                nc.tensor.matmul(
                                    ot[(h2, jq)][:, q0g - 512 * jq_g:512],
                                    lhsT=r32(vaug_s[ki][:, 65 * h:65 * h + 65]),
                                    rhs=r32(p_t[:, q0g - 1024 * qh:
                                                512 * (jq_g + 1) - 1024 * qh]),
                                    start=(ki == 0), stop=(ki == 4 * jq_g + 3),
                                    skip_group_check=True)
                    # normalize and store transposed attn output
                    for h2 in range(2):
                        for jq in range(2):
                            jq_g = 2 * qh + jq
                            den_sb = rp.tile([65, 512], F32R, tag="r", name="rt")
                            nc.vector.tensor_copy(den_sb[64:65, :],
                                                  ot[(h2, jq)][64:65, :])
                            rb = psS.tile([64, 512], F32, tag="s0", name="s0")
                            nc.tensor.matmul(rb, lhsT=r32(ones64_s[64:65, :]),
                                             rhs=r32(den_sb[64:65, :]),
                                             start=True, stop=True)
                            rb_sb = tn.tile([64, 512], F32, tag="rbs", name="rbs")
                            nc.vector.reciprocal_approx_fast(out=rb_sb, in_=rb)
                            if h2 == 0:
                                nc.vector.tensor_mul(
                                    out=attnT_s[hp][0:64,
                                                    512 * jq_g:512 * (jq_g + 1)],
                                    in0=ot[(h2, jq)][0:64, :], in1=rb_sb)
                            else:
                                t_n = tn.tile([64, 512], F32R, tag="tn", name="tn")
                                nc.vector.tensor_mul(out=t_n,
                                                     in0=ot[(h2, jq)][0:64, :],
                                                     in1=rb_sb)
                                nc.sync.dma_start(
                                    out=attnT_s[hp][64:128,
                                                    512 * jq_g:512 * (jq_g + 1)],
                                    in_=t_n)

        # ---------------- Phase C: output projection ----------------
        with ExitStack() as cctx:
            psC = cctx.enter_context(tc.tile_pool(name="psC", bufs=4, space="PSUM"))
            ob = cctx.enter_context(tc.tile_pool(name="ob", bufs=4))
            for st in range(KT):
                for ec in range(2):
                    ps = psC.tile([128, 512], F32, tag="psC", name="psC")
                    for dl in range(2):
                        nc.tensor.matmul(
                            ps,
                            lhsT=r32(attnT_s[dl][:, 128 * st:128 * (st + 1)]),
                            rhs=r32(wo_s[dl][:, 512 * ec:512 * (ec + 1)]),
                            start=(dl == 0), stop=(dl == 1))
                    o_t = ob.tile([128, 512], F32, tag="ob", name="ob")
                    nc.vector.tensor_copy(o_t, ps)
                    nc.sync.dma_start(
                        out=out_d[128 * st:128 * (st + 1),
                                  512 * ec:512 * (ec + 1)],
                        in_=o_t)

    nc.compile()
    return nc


def _get_program():
    global _NC
    if _NC is None:
        _NC = _build_program()
    return _NC


def kernel(x, w_qkv, b_qkv, w_out, b_out):
    from concourse.bass_utils import run_bass_kernel_spmd

    x = np.asarray(x, dtype=np.float32)
    w_qkv = np.asarray(w_qkv, dtype=np.float32)
    b_qkv = np.asarray(b_qkv, dtype=np.float32)
    w_out = np.asarray(w_out, dtype=np.float32)
    b_out = np.asarray(b_out, dtype=np.float32)

    nc = _get_program()

    in_maps = []
    for c in range(N_CORES):
        b = c // 4
        g = c % 4
        hs = slice(g * EL, (g + 1) * EL)
        wq = w_qkv[0 * D:1 * D][hs]          # [256, 1024]
        wk = w_qkv[1 * D:2 * D][hs]
        wv = w_qkv[2 * D:3 * D][hs]
        bq = b_qkv[0 * D:1 * D][hs]
        bk = b_qkv[1 * D:2 * D][hs]
        bv = b_qkv[2 * D:3 * D][hs]
        bqk = np.concatenate([bq, bk])       # [512]
        wvx = np.zeros((D, HPC * 65), dtype=np.float32)
        bvx = np.zeros((1, HPC * 65), dtype=np.float32)
        for h in range(HPC):
            wvx[:, 65 * h:65 * h + 64] = wv[h * DH:(h + 1) * DH].T
            bvx[0, 65 * h:65 * h + 64] = bv[h * DH:(h + 1) * DH]
            bvx[0, 65 * h + 64] = 1.0
        in_maps.append({
            "xt": np.ascontiguousarray(x[b].T),                      # [1024, 2048]
            "wqk": np.ascontiguousarray(np.concatenate([wq, wk]).T), # [1024, 512]
            "wv": wvx,                                               # [1024, 260]
            "bqk": np.ascontiguousarray(bqk.reshape(4, 128).T),      # [128, 4]
            "bv": bvx,                                               # [1, 260]
            "ones": np.ones((1, 512), dtype=np.float32),
            "wo": np.ascontiguousarray(w_out[:, hs].T),              # [256, 1024]
        })

    global _last_in_maps
    _last_in_maps = in_maps
    res = run_bass_kernel_spmd(nc, in_maps, list(range(N_CORES)))

    out = np.empty((B, S, D), dtype=np.float32)
    for b in range(B):
        acc = res.results[4 * b]["out"].astype(np.float32)
        for j in range(1, 4):
            acc = acc + res.results[4 * b + j]["out"]
        out[b] = acc + b_out[None, :]
    return out



# revision 2
# speedup vs baseline: 1.1809x; 1.1809x over previous
"""Causal self-attention (B=2, S=2048, D=1024, H=16, Dh=64) on 8 NeuronCores.

Sharding: core c -> batch b = c//4, head-group g = c%4 (heads 4g..4g+3).
Each core computes QKV projection for its 4 heads, causal attention
(scores kept transposed: [k, q] layout so no on-chip transposes are
needed), and a partial output projection over its local head dims.
Host sums the 4 partials per batch and adds b_out.

All matmuls run in bf16 (fp32r tripped the hardware power throttle to
50% PE duty for most of the kernel); accumulation stays fp32 in PSUM.
"""

import numpy as np
from contextlib import ExitStack

B = 2
S = 2048
D = 1024
NH = 16
DH = 64
N_CORES = 8
HPC = 4            # heads per core
EL = HPC * DH      # 256 local head dims per core
KD = D // 128      # 8 contraction chunks for projections
KT = S // 128      # 16 key tiles

_NC = None
_last_in_maps = None


def _build_program():
    import concourse.mybir as mybir
    import concourse.tile as tile
    from concourse import bacc

    F32 = mybir.dt.float32
    BF16 = mybir.dt.bfloat16
    Exp = mybir.ActivationFunctionType.Exp

    nc = bacc.Bacc("TRN2", target_bir_lowering=False, debug=False,
                   num_devices=N_CORES)

    xt_d = nc.dram_tensor("xt", [D, S], BF16, kind="ExternalInput")
    wqk_d = nc.dram_tensor("wqk", [D, 2 * EL], BF16, kind="ExternalInput")
    wv_d = nc.dram_tensor("wv", [D, HPC * 65], BF16, kind="ExternalInput")
    bqk_d = nc.dram_tensor("bqk", [128, 4], F32, kind="ExternalInput")
    ones_d = nc.dram_tensor("ones", [1, 512], BF16, kind="ExternalInput")
    bv_d = nc.dram_tensor("bv", [1, HPC * 65], BF16, kind="ExternalInput")
    wo_d = nc.dram_tensor("wo", [EL, D], BF16, kind="ExternalInput")
    out_d = nc.dram_tensor("out", [S, D], BF16, kind="ExternalOutput")

    with nc.allow_low_precision(reason="bf16 matmul input tensors"), \
         tile.TileContext(nc) as tc, ExitStack() as ctx:
        const = ctx.enter_context(tc.tile_pool(name="const", bufs=1))
        work = ctx.enter_context(tc.tile_pool(name="work", bufs=1))

        wo_s = [const.tile([128, D], BF16, tag=f"wo{i}", name=f"wo{i}") for i in range(2)]
        for i in range(2):
            nc.sync.dma_start(out=wo_s[i], in_=wo_d[128 * i:128 * (i + 1), :])
        bqk_s = const.tile([128, 4], F32, tag="bqk", name="bqk")
        nc.sync.dma_start(out=bqk_s, in_=bqk_d[:, :])
        bv_s = const.tile([1, HPC * 65], BF16, tag="bv", name="bv")
        nc.sync.dma_start(out=bv_s, in_=bv_d[:, :])
        ones_s = const.tile([1, 512], BF16, tag="ones", name="ones")
        nc.sync.dma_start(out=ones_s, in_=ones_d[:, :])
        # ones living on partition 64, for the base-64 denominator matmul
        ones64_s = const.tile([65, 64], BF16, tag="ones64", name="ones64")
        nc.sync.dma_start(out=ones64_s[64:65, :], in_=ones_d[0:1, 0:64])

        # qkT: e-tiles 0,1 = Q (heads 0..3), 2,3 = K
        qk_s = [work.tile([128, S], BF16, tag=f"qk{e}", name=f"qk{e}") for e in range(4)]
        # V augmented: per key-tile [128, 4*65]; col 64 of each head = 1.0
        vaug_s = [work.tile([128, HPC * 65], BF16, tag=f"va{t}", name=f"va{t}") for t in range(KT)]
        # normalized attn output, transposed: [d_local, s]
        attnT_s = [work.tile([128, S], BF16, tag=f"at{d}", name=f"at{d}") for d in range(2)]

        # ---------------- Phase A: projections ----------------
        with ExitStack() as actx:
            pin = actx.enter_context(tc.tile_pool(name="pin", bufs=1))
            psA = actx.enter_context(tc.tile_pool(name="psA", bufs=2, space="PSUM"))
            psV = actx.enter_context(tc.tile_pool(name="psV", bufs=2, space="PSUM"))

            xt_s = [pin.tile([128, S], BF16, tag=f"xt{k}", name=f"xt{k}") for k in range(KD)]
            wqk_s = [pin.tile([128, 2 * EL], BF16, tag=f"wqk{k}", name=f"wqk{k}") for k in range(KD)]
            wv_s = [pin.tile([128, HPC * 65], BF16, tag=f"wv{k}", name=f"wv{k}") for k in range(KD)]
            for k in range(KD):
                nc.sync.dma_start(out=wqk_s[k], in_=wqk_d[128 * k:128 * (k + 1), :])
                nc.sync.dma_start(out=wv_s[k], in_=wv_d[128 * k:128 * (k + 1), :])
                nc.sync.dma_start(out=xt_s[k], in_=xt_d[128 * k:128 * (k + 1), :])

            # qkT[e, s] = sum_d wqk[d, e] * xt[d, s]   (+ bias per-partition)
            for e in range(4):
                for sc in range(4):
                    ps = psA.tile([128, 512], F32, tag="psA", name="psA")
                    for k in range(KD):
                        nc.tensor.matmul(
                            ps,
                            lhsT=wqk_s[k][:, 128 * e:128 * (e + 1)],
                            rhs=xt_s[k][:, 512 * sc:512 * (sc + 1)],
                            start=(k == 0), stop=(k == KD - 1))
                    nc.vector.tensor_scalar_add(
                        out=qk_s[e][:, 512 * sc:512 * (sc + 1)], in0=ps,
                        scalar1=bqk_s[:, e:e + 1])

            # Vaug[s, hc] = sum_d xt[d, s] * wv[d, hc]  (+ bias via ones row;
            # wv col 65h+64 is zero with bias 1.0 -> the softmax-denominator ones)
            for st in range(KT):
                ps = psV.tile([128, HPC * 65], F32, tag="psV", name="psV")
                for k in range(KD):
                    nc.tensor.matmul(
                        ps,
                        lhsT=xt_s[k][:, 128 * st:128 * (st + 1)],
                        rhs=wv_s[k],
                        start=(k == 0), stop=False)
                nc.tensor.matmul(ps, lhsT=ones_s[0:1, 0:128], rhs=bv_s,
                                 start=False, stop=True)
                nc.vector.tensor_copy(vaug_s[st], ps)

        # ---------------- Phase B: attention ----------------
        # Head pairs: K=64 score matmuls strictly alternate row groups
        # (bases 0/64) so they run concurrently and keep the PE clock warm.
        with ExitStack() as bctx:
            psS = bctx.enter_context(tc.tile_pool(name="psS", bufs=1, space="PSUM"))
            psOT = bctx.enter_context(tc.tile_pool(name="psOT", bufs=1, space="PSUM"))
            pp = bctx.enter_context(tc.tile_pool(name="pp", bufs=3))
            rp = bctx.enter_context(tc.tile_pool(name="rp", bufs=2))
            tn = bctx.enter_context(tc.tile_pool(name="tn", bufs=2))

            for hp in range(2):
                qt = qk_s[hp]
                kt_ = qk_s[2 + hp]
                for qh in range(2):
                    ot = {}
                    for h2 in range(2):
                        for jq in range(2):
                            ot[(h2, jq)] = psOT.tile(
                                [65, 512], F32, tag=f"ot{h2}{jq}", name=f"ot{h2}{jq}")
                    for ki in range(8 * (qh + 1)):
                        s_ps = {}
                        for h2 in range(2):
                            s_ps[h2] = psS.tile([128, 1024], F32,
                                                tag=f"s{h2}", name=f"s{h2}")
                        for qq in range(2):
                            q0 = 1024 * qh + 512 * qq
                            if q0 + 512 <= 128 * ki:
                                continue
                            for h2 in range(2):
                                pb = 64 * h2
                                nc.tensor.matmul(
                                    s_ps[h2][:, 512 * qq:512 * (qq + 1)],
                                    lhsT=kt_[pb:pb + 64, 128 * ki:128 * (ki + 1)],
                                    rhs=qt[pb:pb + 64, q0:q0 + 512],
                                    start=True, stop=True)
                        for h2 in range(2):
                            alo = max(1024 * qh, 128 * ki)
                            p_t = pp.tile([128, 1024], BF16, tag="p", name="pt")
                            nc.scalar.activation(
                                out=p_t[:, alo - 1024 * qh:1024],
                                in_=s_ps[h2][:, alo - 1024 * qh:1024],
                                func=Exp, scale=0.125)
                            if 128 * ki >= 1024 * qh:
                                off = 128 * ki - 1024 * qh
                                nc.gpsimd.affine_select(
                                    out=p_t[:, off:off + 128],
                                    in_=p_t[:, off:off + 128],
                                    compare_op=mybir.AluOpType.is_ge, fill=0.0,
                                    base=0, pattern=[[1, 128]], channel_multiplier=-1)
                            h = 2 * hp + h2
                            for jq in range(2):
                                jq_g = 2 * qh + jq
                                if ki > 4 * jq_g + 3:
                                    continue
                                q0g = max(512 * jq_g, 128 * ki)
                                nc.tensor.matmul(
                                    ot[(h2, jq)][:, q0g - 512 * jq_g:512],
                                    lhsT=vaug_s[ki][:, 65 * h:65 * h + 65],
                                    rhs=p_t[:, q0g - 1024 * qh:
                                                512 * (jq_g + 1) - 1024 * qh],
                                    start=(ki == 0), stop=(ki == 4 * jq_g + 3),
                                    skip_group_check=True)
                    # normalize and store transposed attn output
                    for h2 in range(2):
                        for jq in range(2):
                            jq_g = 2 * qh + jq
                            den_sb = rp.tile([65, 512], BF16, tag="r", name="rt")
                            nc.vector.tensor_copy(den_sb[64:65, :],
                                                  ot[(h2, jq)][64:65, :])
                            rb = psS.tile([64, 512], F32, tag="s0", name="s0")
                            nc.tensor.matmul(rb, lhsT=ones64_s[64:65, :],
                                             rhs=den_sb[64:65, :],
                                             start=True, stop=True)
                            rb_sb = tn.tile([64, 512], F32, tag="rbs", name="rbs")
                            nc.vector.reciprocal_approx_fast(out=rb_sb, in_=rb)
                            if h2 == 0:
                                nc.vector.tensor_mul(
                                    out=attnT_s[hp][0:64,
                                                    512 * jq_g:512 * (jq_g + 1)],
                                    in0=ot[(h2, jq)][0:64, :], in1=rb_sb)
                            else:
                                t_n = tn.tile([64, 512], BF16, tag="tn", name="tn")
                                nc.vector.tensor_mul(out=t_n,
                                                     in0=ot[(h2, jq)][0:64, :],
                                                     in1=rb_sb)
                                nc.sync.dma_start(
                                    out=attnT_s[hp][64:128,
                                                    512 * jq_g:512 * (jq_g + 1)],
                                    in_=t_n)

        # ---------------- Phase C: output projection ----------------
        with ExitStack() as cctx:
            psC = cctx.enter_context(tc.tile_pool(name="psC", bufs=4, space="PSUM"))
            ob = cctx.enter_context(tc.tile_pool(name="ob", bufs=4))
            for st in range(KT):
                for ec in range(2):
                    ps = psC.tile([128, 512], F32, tag="psC", name="psC")
                    for dl in range(2):
                        nc.tensor.matmul(
                            ps,
                            lhsT=attnT_s[dl][:, 128 * st:128 * (st + 1)],
                            rhs=wo_s[dl][:, 512 * ec:512 * (ec + 1)],
                            start=(dl == 0), stop=(dl == 1))
                    o_t = ob.tile([128, 512], BF16, tag="ob", name="ob")
                    nc.vector.tensor_copy(o_t, ps)
                    nc.sync.dma_start(
                        out=out_d[128 * st:128 * (st + 1),
                                  512 * ec:512 * (ec + 1)],
                        in_=o_t)

    nc.compile()
    return nc


def _get_program():
    global _NC
    if _NC is None:
        _NC = _build_program()
    return _NC


def kernel(x, w_qkv, b_qkv, w_out, b_out):
    import ml_dtypes
    from concourse.bass_utils import run_bass_kernel_spmd

    BF = ml_dtypes.bfloat16
    x = np.asarray(x, dtype=np.float32)
    w_qkv = np.asarray(w_qkv, dtype=np.float32)
    b_qkv = np.asarray(b_qkv, dtype=np.float32)
    w_out = np.asarray(w_out, dtype=np.float32)
    b_out = np.asarray(b_out, dtype=np.float32)

    nc = _get_program()

    in_maps = []
    for c in range(N_CORES):
        b = c // 4
        g = c % 4
        hs = slice(g * EL, (g + 1) * EL)
        wq = w_qkv[0 * D:1 * D][hs]          # [256, 1024]
        wk = w_qkv[1 * D:2 * D][hs]
        wv = w_qkv[2 * D:3 * D][hs]
        bq = b_qkv[0 * D:1 * D][hs]
        bk = b_qkv[1 * D:2 * D][hs]
        bv = b_qkv[2 * D:3 * D][hs]
        bqk = np.concatenate([bq, bk])       # [512]
        wvx = np.zeros((D, HPC * 65), dtype=np.float32)
        bvx = np.zeros((1, HPC * 65), dtype=np.float32)
        for h in range(HPC):
            wvx[:, 65 * h:65 * h + 64] = wv[h * DH:(h + 1) * DH].T
            bvx[0, 65 * h:65 * h + 64] = bv[h * DH:(h + 1) * DH]
            bvx[0, 65 * h + 64] = 1.0
        in_maps.append({
            "xt": np.ascontiguousarray(x[b].T).astype(BF),               # [1024, 2048]
            "wqk": np.ascontiguousarray(np.concatenate([wq, wk]).T).astype(BF),
            "wv": wvx.astype(BF),                                        # [1024, 260]
            "bqk": np.ascontiguousarray(bqk.reshape(4, 128).T),          # [128, 4]
            "bv": bvx.astype(BF),                                        # [1, 260]
            "ones": np.ones((1, 512), dtype=BF),
            "wo": np.ascontiguousarray(w_out[:, hs].T).astype(BF),       # [256, 1024]
        })

    global _last_in_maps
    _last_in_maps = in_maps
    res = run_bass_kernel_spmd(nc, in_maps, list(range(N_CORES)))

    out = np.empty((B, S, D), dtype=np.float32)
    for b in range(B):
        acc = res.results[4 * b]["out"].astype(np.float32)
        for j in range(1, 4):
            acc = acc + res.results[4 * b + j]["out"].astype(np.float32)
        out[b] = acc + b_out[None, :]
    return out


# revision 4
# speedup vs baseline: 1.6468x; 1.3945x over previous
"""Causal self-attention (B=2, S=2048, D=1024, H=16, Dh=64) on 8 NeuronCores.

Sharding: core c -> batch b = c//4, head-group g = c%4 (heads 4g..4g+3).
Each core computes QKV projection for its 4 heads, causal attention
(scores kept transposed: [k, q] layout so no on-chip transposes are
needed), and a partial output projection over its local head dims.
Host sums the 4 partials per batch and adds b_out.

All matmuls run in bf16 (fp32r tripped the hardware power throttle);
accumulation stays fp32 in PSUM.  Structure: q-blocks of 512 with both
heads of a pair packed into one [128,2,512] score PSUM tile (one wide
exp per (block, ki) keeps Act-engine instruction overhead low); PV
matmuls lag one ki behind scores so PE never stalls on exp; projection
and output-projection rounds are interleaved into the attention ki
loops to fill PE gaps and shrink the head/tail.
"""

import numpy as np
from contextlib import ExitStack

B = 2
S = 2048
D = 1024
NH = 16
DH = 64
N_CORES = 8
HPC = 4            # heads per core
EL = HPC * DH      # 256 local head dims per core
KD = D // 128      # 8 contraction chunks for projections
KT = S // 128      # 16 key tiles

_NC = None
_last_in_maps = None


def _build_program():
    import concourse.mybir as mybir
    import concourse.tile as tile
    from concourse import bacc

    F32 = mybir.dt.float32
    BF16 = mybir.dt.bfloat16
    Exp = mybir.ActivationFunctionType.Exp

    nc = bacc.Bacc("TRN2", target_bir_lowering=False, debug=False,
                   num_devices=N_CORES)

    xt_d = nc.dram_tensor("xt", [D, S], BF16, kind="ExternalInput")
    wqk_d = nc.dram_tensor("wqk", [D, 2 * EL], BF16, kind="ExternalInput")
    wv_d = nc.dram_tensor("wv", [D, HPC * 65], BF16, kind="ExternalInput")
    bqk_d = nc.dram_tensor("bqk", [128, 4], F32, kind="ExternalInput")
    ones_d = nc.dram_tensor("ones", [1, 512], BF16, kind="ExternalInput")
    bv_d = nc.dram_tensor("bv", [1, HPC * 65], BF16, kind="ExternalInput")
    wo_d = nc.dram_tensor("wo", [EL, D], BF16, kind="ExternalInput")
    out_d = nc.dram_tensor("out", [S, D], BF16, kind="ExternalOutput")

    with nc.allow_low_precision(reason="bf16 matmul input tensors"), \
         tile.TileContext(nc) as tc, ExitStack() as ctx:
        const = ctx.enter_context(tc.tile_pool(name="const", bufs=1))
        work = ctx.enter_context(tc.tile_pool(name="work", bufs=1))

        # attention-phase psum pools (live for the whole kernel):
        #   psS: tag "s" bufs=1 -> [128,2,512] f32 = 2 banks
        #   psOT: tags ot0/ot1 bufs=2 -> 4 banks ("rb" broadcasts borrow slots)
        psS = ctx.enter_context(tc.tile_pool(name="psS", bufs=1, space="PSUM"))
        psOT = ctx.enter_context(tc.tile_pool(name="psOT", bufs=2, space="PSUM"))
        pp = ctx.enter_context(tc.tile_pool(name="pp", bufs=3))
        rp = ctx.enter_context(tc.tile_pool(name="rp", bufs=2))
        tn = ctx.enter_context(tc.tile_pool(name="tn", bufs=2))

        # --- inputs: xt + wqk first (projections gate everything) ---
        pin = ctx.enter_context(tc.tile_pool(name="pin", bufs=1))
        xt_s = [pin.tile([128, S], BF16, tag=f"xt{k}", name=f"xt{k}") for k in range(KD)]
        wqk_s = [pin.tile([128, 2 * EL], BF16, tag=f"wqk{k}", name=f"wqk{k}") for k in range(KD)]
        wv_s = [pin.tile([128, HPC * 65], BF16, tag=f"wv{k}", name=f"wv{k}") for k in range(KD)]
        for k in range(KD):
            nc.sync.dma_start(out=xt_s[k], in_=xt_d[128 * k:128 * (k + 1), :])
            nc.sync.dma_start(out=wqk_s[k], in_=wqk_d[128 * k:128 * (k + 1), :])
        bqk_s = const.tile([128, 4], F32, tag="bqk", name="bqk")
        nc.sync.dma_start(out=bqk_s, in_=bqk_d[:, :])
        bv_s = const.tile([1, HPC * 65], BF16, tag="bv", name="bv")
        nc.sync.dma_start(out=bv_s, in_=bv_d[:, :])
        ones_s = const.tile([1, 512], BF16, tag="ones", name="ones")
        nc.sync.dma_start(out=ones_s, in_=ones_d[:, :])
        ones64_s = const.tile([65, 64], BF16, tag="ones64", name="ones64")
        nc.sync.dma_start(out=ones64_s[64:65, :], in_=ones_d[0:1, 0:64])
        for k in range(KD):
            nc.sync.dma_start(out=wv_s[k], in_=wv_d[128 * k:128 * (k + 1), :])
        wo_s = [const.tile([128, D], BF16, tag=f"wo{i}", name=f"wo{i}") for i in range(2)]
        for i in range(2):
            nc.sync.dma_start(out=wo_s[i], in_=wo_d[128 * i:128 * (i + 1), :])

        # qkT: e-tiles 0,1 = Q (head pairs 0,1), 2,3 = K
        qk_s = [work.tile([128, S], BF16, tag=f"qk{e}", name=f"qk{e}") for e in range(4)]
        # V augmented: per key-tile [128, 4*65]; col 64 of each head = 1.0
        vaug_s = [work.tile([128, HPC * 65], BF16, tag=f"va{t}", name=f"va{t}") for t in range(KT)]
        # normalized attn output, transposed: [d_local, s]
        attnT_s = [work.tile([128, S], BF16, tag=f"at{d}", name=f"at{d}") for d in range(2)]

        # ---------------- emission helpers ----------------
        psProj_ctx = ExitStack()
        psProj = psProj_ctx.enter_context(
            tc.tile_pool(name="psProj", bufs=2, space="PSUM"))

        def proj_qk_group(e, sc):
            ps = psProj.tile([128, 512], F32, tag="pj", name="pj")
            for k in range(KD):
                nc.tensor.matmul(
                    ps,
                    lhsT=wqk_s[k][:, 128 * e:128 * (e + 1)],
                    rhs=xt_s[k][:, 512 * sc:512 * (sc + 1)],
                    start=(k == 0), stop=(k == KD - 1))
            nc.vector.tensor_scalar_add(
                out=qk_s[e][:, 512 * sc:512 * (sc + 1)], in0=ps,
                scalar1=bqk_s[:, e:e + 1])

        def proj_v_group(st):
            ps = psProj.tile([128, HPC * 65], F32, tag="pj", name="pjv")
            for k in range(KD):
                nc.tensor.matmul(
                    ps,
                    lhsT=xt_s[k][:, 128 * st:128 * (st + 1)],
                    rhs=wv_s[k],
                    start=(k == 0), stop=False)
            nc.tensor.matmul(ps, lhsT=ones_s[0:1, 0:128], rhs=bv_s,
                             start=False, stop=True)
            nc.vector.tensor_copy(vaug_s[st], ps)

        def proj_block(sc):
            for e in (2, 0, 3, 1):
                proj_qk_group(e, sc)
            for st in range(4 * sc, 4 * sc + 4):
                proj_v_group(st)

        # deferred PE-heavy work units, drained one per ki iteration
        fill_q = []

        def attn_block(hp, jq):
            """Causal attention for head pair hp, q block [512*jq, 512*(jq+1))."""
            qt = qk_s[hp]
            kt_ = qk_s[2 + hp]
            q0 = 512 * jq
            ki_max = 4 * jq + 3
            ot = [psOT.tile([65, 512], F32, tag=f"ot{h2}", name=f"ot{h2}")
                  for h2 in range(2)]
            pend = None  # (ki, p_t) awaiting PV emission
            for ki in range(ki_max + 1):
                alo = max(q0, 128 * ki) - q0
                s_ps = psS.tile([128, 2, 512], F32, tag="s", name="s")
                for h2 in range(2):
                    nc.tensor.matmul(
                        s_ps[:, h2, alo:512],
                        lhsT=kt_[64 * h2:64 * h2 + 64, 128 * ki:128 * (ki + 1)],
                        rhs=qt[64 * h2:64 * h2 + 64, q0 + alo:q0 + 512],
                        start=True, stop=True)
                p_t = pp.tile([128, 2, 512], BF16, tag="p", name="pt")
                nc.scalar.activation(
                    out=p_t[:, :, alo:512], in_=s_ps[:, :, alo:512],
                    func=Exp, scale=0.125)
                if 128 * ki >= q0:
                    off = 128 * ki - q0
                    for h2 in range(2):
                        nc.gpsimd.affine_select(
                            out=p_t[:, h2, off:off + 128],
                            in_=p_t[:, h2, off:off + 128],
                            compare_op=mybir.AluOpType.is_ge, fill=0.0,
                            base=0, pattern=[[1, 128]], channel_multiplier=-1)
                if pend is not None:
                    emit_pv(hp, jq, ot, *pend)
                pend = (ki, p_t, alo)
                if fill_q:
                    fill_q.pop(0)()
            emit_pv(hp, jq, ot, *pend)
            # normalize and store transposed attn output
            for h2 in range(2):
                den_sb = rp.tile([65, 512], BF16, tag="r", name="rt")
                nc.vector.tensor_copy(den_sb[64:65, :], ot[h2][64:65, :])
                rb = psOT.tile([64, 512], F32, tag=f"ot{h2}", name=f"rb{h2}")
                nc.tensor.matmul(rb, lhsT=ones64_s[64:65, :],
                                 rhs=den_sb[64:65, :],
                                 start=True, stop=True)
                rb_sb = tn.tile([64, 512], F32, tag="rbs", name="rbs")
                nc.vector.reciprocal_approx_fast(out=rb_sb, in_=rb)
                if h2 == 0:
                    nc.vector.tensor_mul(
                        out=attnT_s[hp][0:64, q0:q0 + 512],
                        in0=ot[h2][0:64, :], in1=rb_sb)
                else:
                    t_n = tn.tile([64, 512], BF16, tag="tn", name="tn")
                    nc.vector.tensor_mul(out=t_n, in0=ot[h2][0:64, :],
                                         in1=rb_sb)
                    nc.sync.dma_start(
                        out=attnT_s[hp][64:128, q0:q0 + 512], in_=t_n)

        def emit_pv(hp, jq, ot, ki, p_t, alo):
            ki_max = 4 * jq + 3
            for h2 in range(2):
                h = 2 * hp + h2
                nc.tensor.matmul(
                    ot[h2][:, alo:512],
                    lhsT=vaug_s[ki][:, 65 * h:65 * h + 65],
                    rhs=p_t[:, h2, alo:512],
                    start=(ki == 0), stop=(ki == ki_max),
                    skip_group_check=True)

        # ---------------- emission schedule ----------------
        proj_block(0)
        proj_block(1)
        attn_block(0, 0)
        attn_block(1, 0)
        # spread P2's 8 groups into the 16 ki-iterations of A(*,1)
        fill_q.extend([lambda e=e: proj_qk_group(e, 2) for e in (2, 0, 3, 1)])
        fill_q.extend([lambda st=st: proj_v_group(st) for st in range(8, 12)])
        attn_block(0, 1)
        attn_block(1, 1)
        while fill_q:
            fill_q.pop(0)()
        # spread P3 into A(*,2)
        fill_q.extend([lambda e=e: proj_qk_group(e, 3) for e in (2, 0, 3, 1)])
        fill_q.extend([lambda st=st: proj_v_group(st) for st in range(12, 16)])
        attn_block(0, 2)
        attn_block(1, 2)
        while fill_q:
            fill_q.pop(0)()
        psProj_ctx.close()

        # output projection: rounds for q-block jq need attnT cols from both
        # head pairs, ready after attn_block(1, jq)
        psC_ctx = ExitStack()
        psC = psC_ctx.enter_context(tc.tile_pool(name="psC", bufs=2, space="PSUM"))
        ob = psC_ctx.enter_context(tc.tile_pool(name="ob", bufs=3))

        def op_round(st, ec):
            ps = psC.tile([128, 512], F32, tag="psC", name="psC")
            for dl in range(2):
                nc.tensor.matmul(
                    ps,
                    lhsT=attnT_s[dl][:, 128 * st:128 * (st + 1)],
                    rhs=wo_s[dl][:, 512 * ec:512 * (ec + 1)],
                    start=(dl == 0), stop=(dl == 1))
            o_t = ob.tile([128, 512], BF16, tag="ob", name="ob")
            nc.vector.tensor_copy(o_t, ps)
            nc.sync.dma_start(
                out=out_d[128 * st:128 * (st + 1), 512 * ec:512 * (ec + 1)],
                in_=o_t)

        # OP rounds for jq 0,1 spread into A(*,3); jq 2,3 run as tail
        fill_q.extend([lambda st=st, ec=ec: op_round(st, ec)
                       for st in range(0, 8) for ec in range(2)])
        attn_block(0, 3)
        attn_block(1, 3)
        while fill_q:
            fill_q.pop(0)()
        for st in range(8, 16):
            for ec in range(2):
                op_round(st, ec)
        psC_ctx.close()

    nc.compile()
    return nc


def _get_program():
    global _NC
    if _NC is None:
        _NC = _build_program()
    return _NC


def kernel(x, w_qkv, b_qkv, w_out, b_out):
    import ml_dtypes
    from concourse.bass_utils import run_bass_kernel_spmd

    BF = ml_dtypes.bfloat16
    x = np.asarray(x, dtype=np.float32)
    w_qkv = np.asarray(w_qkv, dtype=np.float32)
    b_qkv = np.asarray(b_qkv, dtype=np.float32)
    w_out = np.asarray(w_out, dtype=np.float32)
    b_out = np.asarray(b_out, dtype=np.float32)

    nc = _get_program()

    in_maps = []
    for c in range(N_CORES):
        b = c // 4
        g = c % 4
        hs = slice(g * EL, (g + 1) * EL)
        wq = w_qkv[0 * D:1 * D][hs]          # [256, 1024]
        wk = w_qkv[1 * D:2 * D][hs]
        wv = w_qkv[2 * D:3 * D][hs]
        bq = b_qkv[0 * D:1 * D][hs]
        bk = b_qkv[1 * D:2 * D][hs]
        bv = b_qkv[2 * D:3 * D][hs]
        bqk = np.concatenate([bq, bk])       # [512]
        wvx = np.zeros((D, HPC * 65), dtype=np.float32)
        bvx = np.zeros((1, HPC * 65), dtype=np.float32)
        for h in range(HPC):
            wvx[:, 65 * h:65 * h + 64] = wv[h * DH:(h + 1) * DH].T
            bvx[0, 65 * h:65 * h + 64] = bv[h * DH:(h + 1) * DH]
            bvx[0, 65 * h + 64] = 1.0
        in_maps.append({
            "xt": np.ascontiguousarray(x[b].T).astype(BF),               # [1024, 2048]
            "wqk": np.ascontiguousarray(np.concatenate([wq, wk]).T).astype(BF),
            "wv": wvx.astype(BF),                                        # [1024, 260]
            "bqk": np.ascontiguousarray(bqk.reshape(4, 128).T),          # [128, 4]
            "bv": bvx.astype(BF),                                        # [1, 260]
            "ones": np.ones((1, 512), dtype=BF),
            "wo": np.ascontiguousarray(w_out[:, hs].T).astype(BF),       # [256, 1024]
        })

    global _last_in_maps
    _last_in_maps = in_maps
    res = run_bass_kernel_spmd(nc, in_maps, list(range(N_CORES)))

    out = np.empty((B, S, D), dtype=np.float32)
    for b in range(B):
        acc = res.results[4 * b]["out"].astype(np.float32)
        for j in range(1, 4):
            acc = acc + res.results[4 * b + j]["out"].astype(np.float32)
        out[b] = acc + b_out[None, :]
    return out
